# revision 1
# baseline (speedup 1.0000x reference)
"""Trainium2 Bass kernel for nn_MemoryGate (product-key memory gate, top-32).

Computes, for x:[4,2048,2048], W:[512,2048], keys:[2,1024,256]:
    q = x @ W.T                      -> [B,S,512]
    s1 = q[...,:256] @ keys[0].T     -> [B,S,1024]
    s2 = q[...,256:] @ keys[1].T     -> [B,S,1024]
    (ts1,ti1) = top32(s1); (ts2,ti2) = top32(s2)
    combined = ts1[:,None]+ts2[None,:]  (32x32)
    top32 of combined -> indices ti1*1024+ti2, softmax(scores)

Sharding: tokens (B*S = 8192) split across 8 cores, 1024 per core.
W / keys replicated. Everything else on-chip per core.

Top-32 on the DVE via 4 rounds of max8/max_index with is_ge-mask
elimination (survivor values stay bit-exact). The 32x32 cartesian stage is
reduced to the 119-cell staircase {(i,j): (i+1)(j+1) <= 32} which provably
contains the top-32 of the pair sums. Final index retrieval via indirect
DMA gather from a per-tile DRAM table of combined indices.
"""
import numpy as np

import concourse.bass as bass
import concourse.bacc as bacc
import concourse.mybir as mybir
from concourse.tile import TileContext
from concourse import bass_utils

N_CORES = 8
B, S, DIM = 4, 2048, 2048
KDIM, NKEYS, NC = 512, 1024, 32
HALF = KDIM // 2  # 256
import os
TOK = int(os.environ.get("KERNEL_TOK", (B * S) // N_CORES))  # tokens per core
ITERS = int(os.environ.get("KERNEL_ITERS", "1"))
P = 128                        # partitions / tokens per tile
NTILES = TOK // P              # 8
BLK = 512                      # tokens per matmul block (PSUM free dim)
NBLK = TOK // BLK              # 2
TPB = BLK // P                 # tiles per block = 4
KT = DIM // P                  # 16 K-tiles for the queries matmul
NCHUNK = KDIM // P             # 4 kdim chunks
F32 = mybir.dt.float32
I32 = mybir.dt.int32
U32 = mybir.dt.uint32
NEG_BIG = 2.0e30

# staircase: cells (i, j) with (i+1)*(j+1) <= 32, i-major order
_J = [32 // (i + 1) for i in range(NC)]
NCELL = sum(_J)  # 119
SC_PAD = 128
# runs of equal J: list of (i0, run_len, J)
_RUNS = []
_i = 0
while _i < NC:
    j = _J[_i]
    i0 = _i
    while _i < NC and _J[_i] == j:
        _i += 1
    _RUNS.append((i0, _i - i0, j))


def _build_bass():
    nc = bacc.Bacc("TRN2", target_bir_lowering=False, debug=False,
                   num_devices=N_CORES)

    xt = nc.dram_tensor("xt", [DIM, TOK], F32, kind="ExternalInput").ap()
    wt = nc.dram_tensor("wt", [DIM, KDIM], F32, kind="ExternalInput").ap()
    kt = nc.dram_tensor("kt", [2, HALF, NKEYS], F32, kind="ExternalInput").ap()
    out_idx = nc.dram_tensor("out_idx", [NTILES, P * NC], I32,
                             kind="ExternalOutput").ap()
    out_p = nc.dram_tensor("out_p", [TOK, NC], F32, kind="ExternalOutput").ap()

    ic_dram = [nc.dram_tensor(f"ic_dram{t}", [P * SC_PAD], I32).ap()
               for t in range(NTILES)]
    offs_dram = [nc.dram_tensor(f"offs_dram{t}", [P * NC], I32).ap()
                 for t in range(NTILES)]

    with TileContext(nc) as tc:
        with (
            tc.tile_pool(name="res", bufs=1) as res_pool,
            tc.tile_pool(name="xs", bufs=4) as x_pool,
            tc.tile_pool(name="qs", bufs=2) as q_pool,
            tc.tile_pool(name="ss", bufs=2) as s_pool,
            tc.tile_pool(name="wk", bufs=3) as wk_pool,
            tc.tile_pool(name="sm", bufs=2) as sm_pool,
            tc.tile_pool(name="ps", bufs=1, space="PSUM") as psum_pool,
        ):
            # resident: W^T tiles [128, 16*512], keys [128, 4*1024]
            wt_sb = res_pool.tile([P, KT * KDIM], F32)
            nc.sync.dma_start(
                wt_sb[:].rearrange("p (k n) -> p k n", n=KDIM),
                wt.rearrange("(k p) n -> p k n", p=P),
            )
            k_sb = res_pool.tile([P, 4 * NKEYS], F32)
            for h in range(2):
                for kk in range(2):
                    ch = h * 2 + kk
                    nc.sync.dma_start(
                        k_sb[:, ch * NKEYS:(ch + 1) * NKEYS],
                        kt[h, kk * P:(kk + 1) * P, :],
                    )
            # rowbase[p] = p * SC_PAD  (fp32)
            rowbase_i = res_pool.tile([P, 1], I32)
            nc.gpsimd.iota(rowbase_i[:], pattern=[[0, 1]], base=0,
                           channel_multiplier=SC_PAD)
            rowbase_f = res_pool.tile([P, 1], F32)
            nc.gpsimd.tensor_copy(rowbase_f[:], rowbase_i[:])

            for _it in range(ITERS):
              for blk in range(NBLK):
                  # ---- queries^T for this block: [512 kdim rows, 512 tokens]
                  qpsum = psum_pool.tile([P, NCHUNK * BLK], F32, tag="qpsum")
                  for k in range(KT):
                      xk = x_pool.tile([P, BLK], F32, tag="xk")
                      nc.sync.dma_start(
                          xk[:], xt[k * P:(k + 1) * P, blk * BLK:(blk + 1) * BLK])
                      for c in range(NCHUNK):
                          nc.tensor.matmul(
                              qpsum[:, c * BLK:(c + 1) * BLK],
                              lhsT=wt_sb[:, k * KDIM + c * P:k * KDIM + (c + 1) * P],
                              rhs=xk[:],
                              start=(k == 0), stop=(k == KT - 1),
                          )
                  q_sb = q_pool.tile([P, NCHUNK * BLK], F32, tag="qsb")
                  for c in range(NCHUNK):
                      nc.scalar.activation(
                          q_sb[:, c * BLK:(c + 1) * BLK],
                          qpsum[:, c * BLK:(c + 1) * BLK],
                          mybir.ActivationFunctionType.Copy)

                  for tt in range(TPB):
                      t = blk * TPB + tt
                      # ---- scores: s1, s2 [128 tok, 1024 keys]
                      s_sb = []
                      for h in range(2):
                          spsum = psum_pool.tile([P, NKEYS], F32, tag=f"spsum{h}")
                          for kk in range(2):
                              ch = h * 2 + kk
                              lhsT = q_sb[:, ch * BLK + tt * P:ch * BLK + (tt + 1) * P]
                              for n in range(2):
                                  nc.tensor.matmul(
                                      spsum[:, n * BLK:(n + 1) * BLK],
                                      lhsT=lhsT,
                                      rhs=k_sb[:, (h * 2 + kk) * NKEYS + n * BLK:
                                               (h * 2 + kk) * NKEYS + (n + 1) * BLK],
                                      start=(kk == 0), stop=(kk == 1),
                                  )
                          ssb = s_pool.tile([P, NKEYS], F32, tag=f"s{h}")
                          nc.scalar.activation(ssb[:], spsum[:],
                                               mybir.ActivationFunctionType.Copy)
                          s_sb.append(ssb)

                      # ---- top-32 of each half
                      vals, idxs = [], []
                      for h in range(2):
                          v = sm_pool.tile([P, NC], F32, tag=f"v{h}")
                          ix = sm_pool.tile([P, NC], U32, tag=f"ix{h}")
                          cur = s_sb[h]
                          for r in range(4):
                              v8 = v[:, r * 8:(r + 1) * 8]
                              nc.vector.max(out=v8, in_=cur[:])
                              nc.vector.max_index(out=ix[:, r * 8:(r + 1) * 8],
                                                  in_max=v8, in_values=s_sb[h][:])
                              if r < 3:
                                  m = wk_pool.tile([P, NKEYS], F32, tag="mask")
                                  nc.gpsimd.tensor_scalar(
                                      out=m[:], in0=s_sb[h][:],
                                      scalar1=v[:, r * 8 + 7:r * 8 + 8],
                                      scalar2=NEG_BIG,
                                      op0=mybir.AluOpType.is_ge,
                                      op1=mybir.AluOpType.mult)
                                  bwk = wk_pool.tile([P, NKEYS], F32, tag="bwork")
                                  nc.gpsimd.tensor_tensor(
                                      out=bwk[:], in0=s_sb[h][:], in1=m[:],
                                      op=mybir.AluOpType.subtract)
                                  cur = bwk
                          vals.append(v)
                          idxs.append(ix)

                      # ---- staircase sums s3[tok, 128] and index table ic
                      s3 = sm_pool.tile([P, SC_PAD], F32, tag="s3")
                      nc.gpsimd.memset(s3[:, NCELL:], -3.0e38)
                      ic_f = sm_pool.tile([P, SC_PAD], F32, tag="icf")
                      t1s = sm_pool.tile([P, NC], F32, tag="t1s")
                      t2f = sm_pool.tile([P, NC], F32, tag="t2f")
                      nc.gpsimd.tensor_copy(t2f[:], idxs[1][:])
                      nc.gpsimd.tensor_scalar(
                          out=t1s[:], in0=idxs[0][:], scalar1=float(NKEYS),
                          scalar2=None, op0=mybir.AluOpType.mult)
                      base = 0
                      for (i0, ln, j) in _RUNS:
                          w = ln * j
                          for (dst, a, bsrc) in ((s3, vals[0], vals[1]),
                                                 (ic_f, t1s, t2f)):
                              o3 = dst[:, base:base + w].rearrange(
                                  "p (i j) -> p i j", j=j)
                              a3 = a[:, i0:i0 + ln].rearrange(
                                  "p (i one) -> p i one", one=1).to_broadcast(
                                  [P, ln, j])
                              b3 = bsrc[:, 0:j].rearrange(
                                  "p (one j) -> p one j", one=1).to_broadcast(
                                  [P, ln, j])
                              nc.gpsimd.tensor_tensor(out=o3, in0=a3, in1=b3,
                                                      op=mybir.AluOpType.add)
                          base += w
                      ic_i = sm_pool.tile([P, SC_PAD], I32, tag="ici")
                      nc.gpsimd.tensor_copy(ic_i[:], ic_f[:])
                      nc.sync.dma_start(
                          ic_dram[t][:].rearrange("(p c) -> p c", p=P), ic_i[:])

                      # ---- top-32 of staircase
                      v3 = sm_pool.tile([P, NC], F32, tag="v3")
                      p3 = sm_pool.tile([P, NC], U32, tag="p3")
                      cur = s3
                      for r in range(4):
                          v8 = v3[:, r * 8:(r + 1) * 8]
                          nc.vector.max(out=v8, in_=cur[:])
                          nc.vector.max_index(out=p3[:, r * 8:(r + 1) * 8],
                                              in_max=v8, in_values=s3[:])
                          if r < 3:
                              m3 = wk_pool.tile([P, SC_PAD], F32, tag="mask3")
                              nc.gpsimd.tensor_scalar(
                                  out=m3[:], in0=s3[:],
                                  scalar1=v3[:, r * 8 + 7:r * 8 + 8],
                                  scalar2=NEG_BIG,
                                  op0=mybir.AluOpType.is_ge,
                                  op1=mybir.AluOpType.mult)
                              b3w = wk_pool.tile([P, SC_PAD], F32, tag="b3w")
                              nc.gpsimd.tensor_tensor(
                                  out=b3w[:], in0=s3[:], in1=m3[:],
                                  op=mybir.AluOpType.subtract)
                              cur = b3w

                      # ---- winner positions -> flat offsets into ic_dram[t]
                      p3f = sm_pool.tile([P, NC], F32, tag="p3f")
                      nc.gpsimd.tensor_copy(p3f[:], p3[:])
                      gf = sm_pool.tile([P, NC], F32, tag="gf")
                      nc.gpsimd.tensor_scalar(
                          out=gf[:], in0=p3f[:], scalar1=rowbase_f[:, 0:1],
                          scalar2=None, op0=mybir.AluOpType.add)
                      gi = sm_pool.tile([P, NC], I32, tag="gi")
                      nc.gpsimd.tensor_copy(gi[:], gf[:])
                      nc.sync.dma_start(
                          offs_dram[t][:].rearrange("(p r) -> p r", p=P), gi[:])
                      O = sm_pool.tile([P, NC], I32, tag="O")
                      nc.sync.dma_start(
                          O[:], offs_dram[t][:].rearrange("(b a) -> a b", a=P))
                      gat = sm_pool.tile([1, P * NC], I32, tag="gat")
                      nc.gpsimd.indirect_dma_start(
                          out=gat[:].rearrange("p (r one) -> p r one", one=1),
                          out_offset=None,
                          in_=ic_dram[t][:, None],
                          in_offset=bass.IndirectOffsetOnAxis(ap=O[:], axis=0),
                      )
                      nc.sync.dma_start(out_idx[t:t + 1, :], gat[:])

                      # ---- softmax of v3
                      negmax = sm_pool.tile([P, 1], F32, tag="negmax")
                      nc.vector.tensor_scalar_mul(negmax[:], v3[:, 0:1], -1.0)
                      e = sm_pool.tile([P, NC], F32, tag="esm")
                      nc.scalar.activation(e[:], v3[:],
                                           mybir.ActivationFunctionType.Exp,
                                           bias=negmax[:, 0:1], scale=1.0)
                      ssum = sm_pool.tile([P, 1], F32, tag="ssum")
                      nc.vector.reduce_sum(ssum[:], e[:],
                                           axis=mybir.AxisListType.X)
                      rsum = sm_pool.tile([P, 1], F32, tag="rsum")
                      nc.vector.reciprocal(rsum[:], ssum[:])
                      probs = sm_pool.tile([P, NC], F32, tag="probs")
                      nc.vector.tensor_scalar(
                          out=probs[:], in0=e[:], scalar1=rsum[:, 0:1],
                          scalar2=None, op0=mybir.AluOpType.mult)
                      nc.sync.dma_start(out_p[t * P:(t + 1) * P, :], probs[:])

    nc.compile()
    return nc


_NC_CACHE = None


def _get_bass():
    global _NC_CACHE
    if _NC_CACHE is None:
        _NC_CACHE = _build_bass()
    return _NC_CACHE


def kernel(x, W, keys):
    nc = _get_bass()
    xf = np.ascontiguousarray(x.reshape(B * S, DIM))
    wt_np = np.ascontiguousarray(W.T)                      # [DIM, KDIM]
    kt_np = np.ascontiguousarray(keys.transpose(0, 2, 1))  # [2, HALF, NKEYS]
    in_maps = []
    for c in range(N_CORES):
        shard = xf[c * TOK:(c + 1) * TOK]                  # [TOK, DIM]
        in_maps.append({
            "xt": np.ascontiguousarray(shard.T),           # [DIM, TOK]
            "wt": wt_np,
            "kt": kt_np,
        })
    res = bass_utils.run_bass_kernel_spmd(nc, in_maps,
                                          core_ids=list(range(N_CORES)))
    idx = np.empty((B * S, NC), np.int32)
    pr = np.empty((B * S, NC), np.float32)
    for c in range(N_CORES):
        r = res.results[c]
        idx[c * TOK:(c + 1) * TOK] = r["out_idx"].reshape(TOK, NC)
        pr[c * TOK:(c + 1) * TOK] = r["out_p"].reshape(TOK, NC)
    return idx.reshape(B, S, NC), pr.reshape(B, S, NC)



# revision 3
# speedup vs baseline: 1.0347x; 1.0347x over previous
"""Trainium2 Bass kernel for nn_MemoryGate (product-key memory gate, top-32).

Instruction-count-minimized redesign vs the staged baseline:
  - match_replace for top-k round elimination (1 op vs 2-op masking)
  - staircase sum/index tables built batched across all 8 token tiles
    with 4D broadcast APs (18 ops total instead of 18 per tile)
  - stage-2 winner indices recovered by value-match (eq * ic, row-reduce)
    entirely on-chip -- no DRAM round trip, no indirect DMA, no max_index
  - batched softmax; single-DMA outputs with transposed APs
"""
import numpy as np
import os

import concourse.bass as bass
import concourse.bacc as bacc
import concourse.mybir as mybir
from concourse.tile import TileContext
from concourse import bass_utils

N_CORES = 8
B, S, DIM = 4, 2048, 2048
KDIM, NKEYS, NC = 512, 1024, 32
HALF = KDIM // 2  # 256
TOK = (B * S) // N_CORES       # 1024 tokens per core
ITERS = int(os.environ.get("KERNEL_ITERS", "1"))
P = 128
NTILES = TOK // P              # 8
BLK = 512
NBLK = TOK // BLK              # 2
KT = DIM // P                  # 16
NCHUNK = KDIM // P             # 4
F32 = mybir.dt.float32
I32 = mybir.dt.int32
U32 = mybir.dt.uint32
NEG_BIG = 2.0e30

_J = [32 // (i + 1) for i in range(NC)]
NCELL = sum(_J)  # 119
SC_PAD = 128
_RUNS = []
_i = 0
while _i < NC:
    j = _J[_i]
    i0 = _i
    while _i < NC and _J[_i] == j:
        _i += 1
    _RUNS.append((i0, _i - i0, j))


def _build_bass():
    nc = bacc.Bacc("TRN2", target_bir_lowering=False, debug=False,
                   num_devices=N_CORES)

    xt = nc.dram_tensor("xt", [DIM, TOK], F32, kind="ExternalInput").ap()
    wt = nc.dram_tensor("wt", [DIM, KDIM], F32, kind="ExternalInput").ap()
    kt = nc.dram_tensor("kt", [2, HALF, NKEYS], F32, kind="ExternalInput").ap()
    out_idx = nc.dram_tensor("out_idx", [TOK, NC], I32,
                             kind="ExternalOutput").ap()
    out_p = nc.dram_tensor("out_p", [TOK, NC], F32, kind="ExternalOutput").ap()

    with TileContext(nc) as tc:
        with (
            tc.tile_pool(name="res", bufs=1) as res_pool,
            tc.tile_pool(name="xs", bufs=1) as x_pool,
            tc.tile_pool(name="ss", bufs=2) as s_pool,
            tc.tile_pool(name="sm", bufs=1) as sm_pool,
            tc.tile_pool(name="wk", bufs=1) as wk_pool,
            tc.tile_pool(name="ps", bufs=1, space="PSUM") as psum_pool,
        ):
            # resident: W^T tiles [128, 16*512], keys [128, 4*1024]
            wt_sb = res_pool.tile([P, KT * KDIM], F32)
            nc.sync.dma_start(
                wt_sb[:].rearrange("p (k n) -> p k n", n=KDIM),
                wt.rearrange("(k p) n -> p k n", p=P),
            )
            k_sb = res_pool.tile([P, 4 * NKEYS], F32)
            for h in range(2):
                for kk in range(2):
                    ch = h * 2 + kk
                    nc.sync.dma_start(
                        k_sb[:, ch * NKEYS:(ch + 1) * NKEYS],
                        kt[h, kk * P:(kk + 1) * P, :],
                    )
            # persistent staircase buffers; pads initialized once
            s3_all = res_pool.tile([P, NTILES * SC_PAD], F32)
            ic_all = res_pool.tile([P, NTILES * SC_PAD], F32)
            nc.gpsimd.memset(s3_all[:], -3.0e38)
            nc.gpsimd.memset(ic_all[:], 0.0)
            q_sb = res_pool.tile([P, NCHUNK * TOK], F32)

            for _it in range(ITERS):
                # ---- queries: q^T chunks [4*128 kdim, 1024 tok]
                for blk in range(NBLK):
                    xb = x_pool.tile([P, KT * BLK], F32, tag="xb")
                    nc.sync.dma_start(
                        xb[:].rearrange("p (k t) -> p k t", t=BLK),
                        xt[:, blk * BLK:(blk + 1) * BLK].rearrange(
                            "(k p) t -> p k t", p=P),
                    )
                    qpsum = psum_pool.tile([P, NCHUNK * BLK], F32, tag="qp")
                    for k in range(KT):
                        for c in range(NCHUNK):
                            nc.tensor.matmul(
                                qpsum[:, c * BLK:(c + 1) * BLK],
                                lhsT=wt_sb[:, k * KDIM + c * P:
                                           k * KDIM + (c + 1) * P],
                                rhs=xb[:, k * BLK:(k + 1) * BLK],
                                start=(k == 0), stop=(k == KT - 1),
                            )
                    nc.scalar.activation(
                        q_sb[:].rearrange("p (c t) -> p c t", t=TOK)
                            [:, :, blk * BLK:(blk + 1) * BLK],
                        qpsum[:].rearrange("p (c t) -> p c t", t=BLK),
                        mybir.ActivationFunctionType.Copy)

                v_all = sm_pool.tile([P, 2 * NTILES * NC], F32, tag="vall")
                ti_all = sm_pool.tile([P, 2 * NTILES * NC], U32, tag="tiall")
                v3_all = sm_pool.tile([P, NTILES * NC], F32, tag="v3all")

                # ---- scores + stage-1 top-32 per (tile, half)
                for t in range(NTILES):
                    spsum = psum_pool.tile([P, 2 * NKEYS], F32, tag="sp")
                    for h in range(2):
                        for kk in range(2):
                            lhsT = q_sb[:, (h * 2 + kk) * TOK + t * P:
                                        (h * 2 + kk) * TOK + (t + 1) * P]
                            for n in range(2):
                                nc.tensor.matmul(
                                    spsum[:, h * NKEYS + n * BLK:
                                          h * NKEYS + (n + 1) * BLK],
                                    lhsT=lhsT,
                                    rhs=k_sb[:, (h * 2 + kk) * NKEYS + n * BLK:
                                             (h * 2 + kk) * NKEYS + (n + 1) * BLK],
                                    start=(kk == 0), stop=(kk == 1),
                                )
                    s_sb = s_pool.tile([P, 2 * NKEYS], F32, tag="ssb")
                    nc.scalar.activation(s_sb[:], spsum[:],
                                         mybir.ActivationFunctionType.Copy)
                    for h in range(2):
                        cur = s_sb[:, h * NKEYS:(h + 1) * NKEYS]
                        vbase = (h * NTILES + t) * NC
                        for r in range(4):
                            v8 = v_all[:, vbase + r * 8:vbase + (r + 1) * 8]
                            nc.vector.max(out=v8, in_=cur)
                            nc.vector.max_index(
                                out=ti_all[:, vbase + r * 8:vbase + (r + 1) * 8],
                                in_max=v8, in_values=cur)
                            if r < 3:
                                nc.vector.match_replace(
                                    out=cur, in_to_replace=v8, in_values=cur,
                                    imm_value=-NEG_BIG)

                # ---- index tables as f32: t1s = ti1*1024, t2f = ti2
                tif = sm_pool.tile([P, 2 * NTILES * NC], F32, tag="tif")
                nc.gpsimd.tensor_copy(tif[:], ti_all[:])
                nc.gpsimd.tensor_scalar(
                    out=tif[:, 0:NTILES * NC], in0=tif[:, 0:NTILES * NC],
                    scalar1=float(NKEYS), scalar2=None,
                    op0=mybir.AluOpType.mult)

                # ---- staircase build, batched over all tiles
                s3v = s3_all[:].rearrange("p (t c) -> p t c", c=SC_PAD)
                icv = ic_all[:].rearrange("p (t c) -> p t c", c=SC_PAD)
                v1 = v_all[:, 0:NTILES * NC].rearrange("p (t i) -> p t i", i=NC)
                v2 = v_all[:, NTILES * NC:].rearrange("p (t j) -> p t j", j=NC)
                t1 = tif[:, 0:NTILES * NC].rearrange("p (t i) -> p t i", i=NC)
                t2 = tif[:, NTILES * NC:].rearrange("p (t j) -> p t j", j=NC)
                base = 0
                for (i0, ln, j) in _RUNS:
                    w = ln * j
                    for (dst, a, bsrc) in ((s3v, v1, v2), (icv, t1, t2)):
                        o4 = dst[:, :, base:base + w].rearrange(
                            "p t (i j) -> p t i j", j=j)
                        a4 = a[:, :, i0:i0 + ln].rearrange(
                            "p t (i one) -> p t i one", one=1).to_broadcast(
                            [P, NTILES, ln, j])
                        b4 = bsrc[:, :, 0:j].rearrange(
                            "p t (one j) -> p t one j", one=1).to_broadcast(
                            [P, NTILES, ln, j])
                        nc.gpsimd.tensor_tensor(out=o4, in0=a4, in1=b4,
                                                op=mybir.AluOpType.add)
                    base += w

                # ---- stage-2: destructive rounds on a copy, keep s3 pristine
                s3_keep = wk_pool.tile([P, NTILES * SC_PAD], F32, tag="s3k")
                nc.gpsimd.tensor_copy(s3_keep[:], s3_all[:])
                for t in range(NTILES):
                    cur = s3_keep[:, t * SC_PAD:(t + 1) * SC_PAD]
                    for r in range(4):
                        v8 = v3_all[:, t * NC + r * 8:t * NC + (r + 1) * 8]
                        nc.vector.max(out=v8, in_=cur)
                        if r < 3:
                            nc.vector.match_replace(
                                out=cur, in_to_replace=v8, in_values=cur,
                                imm_value=-3.0e38)

                # ---- winner combined indices: eq-match gather, 2 tiles/chunk
                CT = 2
                cidx = sm_pool.tile([P, NTILES * NC], F32, tag="cidx")
                for cc in range(NTILES // CT):
                    t0 = cc * CT
                    eqw = wk_pool.tile([P, CT * NC * SC_PAD], F32, tag="eqw")
                    e4 = eqw[:].rearrange("p (t r c) -> p t r c",
                                          r=NC, c=SC_PAD)
                    s4 = s3v[:, t0:t0 + CT, :].rearrange(
                        "p t (one c) -> p t one c", one=1).to_broadcast(
                        [P, CT, NC, SC_PAD])
                    w4 = v3_all[:].rearrange("p (t r) -> p t r", r=NC)[
                        :, t0:t0 + CT, :].rearrange(
                        "p t (r one) -> p t r one", one=1).to_broadcast(
                        [P, CT, NC, SC_PAD])
                    nc.vector.tensor_tensor(out=e4, in0=s4, in1=w4,
                                            op=mybir.AluOpType.is_equal)
                    i4 = icv[:, t0:t0 + CT, :].rearrange(
                        "p t (one c) -> p t one c", one=1).to_broadcast(
                        [P, CT, NC, SC_PAD])
                    nc.gpsimd.tensor_tensor(out=e4, in0=e4, in1=i4,
                                            op=mybir.AluOpType.mult)
                    nc.vector.tensor_reduce(
                        out=cidx[:, t0 * NC:(t0 + CT) * NC].rearrange(
                            "p (t r) -> p t r", r=NC),
                        in_=e4, axis=mybir.AxisListType.X,
                        op=mybir.AluOpType.add)
                cidx_i = sm_pool.tile([P, NTILES * NC], I32, tag="cidxi")
                nc.gpsimd.tensor_copy(cidx_i[:], cidx[:])
                nc.sync.dma_start(
                    out_idx.rearrange("(t p) r -> p t r", p=P),
                    cidx_i[:].rearrange("p (t r) -> p t r", r=NC))

                # ---- softmax over v3, batched
                ex = sm_pool.tile([P, NTILES * NC], F32, tag="ex")
                v3v = v3_all[:].rearrange("p (t r) -> p t r", r=NC)
                mx = v3v[:, :, 0:1].to_broadcast([P, NTILES, NC])
                nc.gpsimd.tensor_tensor(
                    out=ex[:].rearrange("p (t r) -> p t r", r=NC),
                    in0=v3v, in1=mx, op=mybir.AluOpType.subtract)
                nc.scalar.activation(ex[:], ex[:],
                                     mybir.ActivationFunctionType.Exp)
                ssum = sm_pool.tile([P, NTILES], F32, tag="ssum")
                nc.vector.tensor_reduce(
                    out=ssum[:], in_=ex[:].rearrange("p (t r) -> p t r", r=NC),
                    axis=mybir.AxisListType.X, op=mybir.AluOpType.add)
                rs = sm_pool.tile([P, NTILES], F32, tag="rs")
                nc.vector.reciprocal(rs[:], ssum[:])
                probs = sm_pool.tile([P, NTILES * NC], F32, tag="probs")
                nc.gpsimd.tensor_tensor(
                    out=probs[:].rearrange("p (t r) -> p t r", r=NC),
                    in0=ex[:].rearrange("p (t r) -> p t r", r=NC),
                    in1=rs[:].rearrange("p (t one) -> p t one", one=1)
                        .to_broadcast([P, NTILES, NC]),
                    op=mybir.AluOpType.mult)
                nc.sync.dma_start(
                    out_p.rearrange("(t p) r -> p t r", p=P),
                    probs[:].rearrange("p (t r) -> p t r", r=NC))

    nc.compile()
    return nc


_NC_CACHE = None


def _get_bass():
    global _NC_CACHE
    if _NC_CACHE is None:
        _NC_CACHE = _build_bass()
    return _NC_CACHE


def kernel(x, W, keys):
    nc = _get_bass()
    xf = np.ascontiguousarray(x.reshape(B * S, DIM))
    wt_np = np.ascontiguousarray(W.T)                      # [DIM, KDIM]
    kt_np = np.ascontiguousarray(keys.transpose(0, 2, 1))  # [2, HALF, NKEYS]
    in_maps = []
    for c in range(N_CORES):
        shard = xf[c * TOK:(c + 1) * TOK]                  # [TOK, DIM]
        in_maps.append({
            "xt": np.ascontiguousarray(shard.T),           # [DIM, TOK]
            "wt": wt_np,
            "kt": kt_np,
        })
    res = bass_utils.run_bass_kernel_spmd(nc, in_maps,
                                          core_ids=list(range(N_CORES)))
    idx = np.empty((B * S, NC), np.int32)
    pr = np.empty((B * S, NC), np.float32)
    for c in range(N_CORES):
        r = res.results[c]
        idx[c * TOK:(c + 1) * TOK] = r["out_idx"].reshape(TOK, NC)
        pr[c * TOK:(c + 1) * TOK] = r["out_p"].reshape(TOK, NC)
    return idx.reshape(B, S, NC), pr.reshape(B, S, NC)
